# revision 39
# baseline (speedup 1.0000x reference)
"""Multi-head attention (RoPE, causal) Trainium2 kernel, 8-way sharded.

Sharding: core c => batch b = c//2, head-group g = c%2 (8 of 16 heads).
Each core computes Q/K/V projections for its (b, g), RoPE, causal
attention over its 8 heads, and the row-slice of the output projection.
Host sums the two partial output projections per batch and adds b_o.

Active path is _build_program_v4 (fp16 matmul operands; ~358us HW):
  - PE streaming and LDWEIGHTS are SBUF-bandwidth-bound on this part
    (~1.35 G rows/s, HAM duty-caps sustained activity to ~50% in bursts),
    so all matmul operands are fp16 (host converts inputs; evictions cast).
  - Q/K projections computed directly transposed (lhsT = W chunk,
    rhs = xT chunk -> psum [d_pair, s]); no PE transposes anywhere.
  - RoPE in [d, s] layout: rotate-half is 4 SBUF->SBUF DMA partition
    copies; then tmp1 = nat*cosT, tmp2 = natr*ssgT (sign folded into the
    table), qT = tmp1+tmp2, all fp16 2x-mode DVE ops.
  - scoresT[k, q] per head via row-paired matmuls (tile_position row
    groups stream concurrently; K = dk = 64). Diagonal blocks trimmed to
    the unmasked column range; fully-masked blocks skipped.
  - softmax without max-subtraction: exp(s/8 - 2) on ACT (bias keeps e^s
    in fp16 range; the factor cancels in normalization). Masked columns
    zeroed by gpsimd memset; the 128-wide triangle masked by a gpsimd
    multiply with a constant fp16 tile.
  - inner loop software-pipelined: QK(kc+1) is emitted between exp(kc)
    and attn@V(kc) so the PE streams scores while ACT runs the exps;
    projection / output-projection units round-robin as fillers.
  - attn @ V with V'' = [V | ones*(1/256) replicated 64x] (M=128): rows
    64:128 accumulate the softmax denominator pre-broadcast across the
    partitions that normalization needs. 1/den = exp(-ln den) on ACT (ln
    and exp share one HW table; custom-DVE recips don't compile here and
    InstReciprocal costs 3.3us). ctx quick-released from PSUM as fp32
    (pre-normalization values can exceed fp16 max), normalized into fp16
    by deferred DVE muls. The 256x scale is divided out of W_o on host.
  - DMA: one trigger per x/w tensor-group (SP trigger cost is ~600ns
    flat, so batching transfers matters more than batching bytes);
    fp16 output partials summed on host.
"""

import json
import os

import numpy as np

# ---------------------------------------------------------------------------
# Workaround: this container's walrus accepts only ONE sync-wait per
# instruction. Hoist every instruction's waits onto single-wait NoOps
# inserted immediately before it (same engine, same program order).
# ---------------------------------------------------------------------------
_PATCHED = False


def _split_multiwait_bir(bir_json: bytes) -> bytes:
    m = json.loads(bir_json)
    ctr = 0
    changed = False
    for f in m.get("functions", []):
        for bl in f.get("blocks", []):
            out = []
            for inst in bl.get("instructions", []):
                si = inst.get("sync_info")
                ow = (si or {}).get("on_wait") or []
                if len(ow) > 1:
                    changed = True
                    for w in ow:
                        ctr += 1
                        out.append({
                            "debug": inst.get("debug", 0),
                            "engine": inst["engine"],
                            "ins": [],
                            "name": f"WSPLIT-{ctr}",
                            "opcode": "NoOp",
                            "outs": [],
                            "sync_info": {"on_update": [], "on_wait": [w]},
                        })
                    si["on_wait"] = []
                out.append(inst)
            if changed:
                bl["instructions"] = out
    if not changed:
        return bir_json
    return json.dumps(m).encode()


def _install_ntff_hook():
    """The agent image's antenv lacks the axon_hooks shim that bass_utils
    imports for trace=True under axon; synthesize it and register the
    ctypes-based NTFF hook from trn_agent_boot (degrades to no-trace if
    anything is missing)."""
    import sys
    import types

    if "antenv.axon_hooks" in sys.modules:
        return
    mod = types.ModuleType("antenv.axon_hooks")
    holder = [None]
    mod.set_axon_ntff_profile_hook = lambda h: holder.__setitem__(0, h)
    mod.get_axon_ntff_profile_hook = lambda: holder[0]
    sys.modules["antenv.axon_hooks"] = mod
    try:
        import antenv
        antenv.axon_hooks = mod
        from trn_agent_boot.trn_boot import _ntff_profile_via_ctypes
        mod.set_axon_ntff_profile_hook(
            _ntff_profile_via_ctypes("/opt/axon/libaxon_pjrt.so"))
    except Exception:
        pass


def _install_patches():
    global _PATCHED
    if _PATCHED:
        return
    import concourse.bass as bass

    orig = bass.Bass.to_json_bytes

    def to_json_bytes_patched(self, *a, **k):
        return _split_multiwait_bir(orig(self, *a, **k))

    bass.Bass.to_json_bytes = to_json_bytes_patched
    _install_ntff_hook()
    _PATCHED = True


# ---------------------------------------------------------------------------
# Problem constants (hardcoded per the harness contract)
# ---------------------------------------------------------------------------
B, S, D = 4, 2048, 1024
H, DK = 16, 64
HG = 8                    # heads per core
DG = HG * DK              # 512: head-group width
N_CORES = 8
ROPE_BASE = 10000.0
P = 128                   # partitions
ST = S // P               # 16 s-tiles
CC = D // P               # 8 contraction chunks for projections
QR = S // 512             # 4 q-ranges of 512
HPAIRS = HG // 2          # 4 head pairs
VSTRIDE = 65              # V columns + ones column (v3)
VPACK = 128               # v4: V columns + 64 replicated ones columns


def _build_program(use_bias: bool, phases: int = 3, mm_dt: str = "fp32"):
    import concourse.bass as bass
    import concourse.mybir as mybir
    import concourse.tile as tile
    from concourse.masks import make_identity

    F32 = mybir.dt.float32
    MDT = mybir.dt.float32r if mm_dt == "fp32r" else F32

    def mmcast(ap):
        return ap

    def dcast(ap):
        # DRAM-side view matching MDT-typed SBUF tiles (bit-identical)
        return ap.bitcast(MDT) if MDT is not F32 else ap
    nc = bass.Bass()

    xqT = nc.dram_tensor("xqT", [D, S], F32, kind="ExternalInput")
    xkT = nc.dram_tensor("xkT", [D, S], F32, kind="ExternalInput")
    xvT = nc.dram_tensor("xvT", [D, S], F32, kind="ExternalInput")
    wqT = nc.dram_tensor("wqT", [D, DG], F32, kind="ExternalInput")
    wkT = nc.dram_tensor("wkT", [D, DG], F32, kind="ExternalInput")
    wvT = nc.dram_tensor("wvT", [D, DG], F32, kind="ExternalInput")
    woT = nc.dram_tensor("woT", [DG, D], F32, kind="ExternalInput")
    cos_d = nc.dram_tensor("cos_d", [S, DK], F32, kind="ExternalInput")
    ssg_d = nc.dram_tensor("ssg_d", [S, DK], F32, kind="ExternalInput")
    if use_bias:
        bias_d = nc.dram_tensor("bias_d", [4, DG], F32, kind="ExternalInput")
        ones_d = nc.dram_tensor("ones_d", [1, P], F32, kind="ExternalInput")
    out_d = nc.dram_tensor("out", [S, D], F32, kind="ExternalOutput")

    with tile.TileContext(nc) as tc:
        with tc.tile_pool(name="consts", bufs=1) as consts, \
             tc.tile_pool(name="xT", bufs=6) as xT_pool, \
             tc.tile_pool(name="w", bufs=8) as w_pool, \
             tc.tile_pool(name="nat", bufs=3) as nat_pool, \
             tc.tile_pool(name="qk", bufs=8) as qk_pool, \
             tc.tile_pool(name="vp", bufs=1) as v_pool, \
             tc.tile_pool(name="ctx", bufs=4) as ctx_pool, \
             tc.tile_pool(name="den", bufs=1) as den_pool, \
             tc.tile_pool(name="w512", bufs=5) as work_pool, \
             tc.tile_pool(name="psm", bufs=4, space="PSUM") as ps_main, \
             tc.tile_pool(name="psc", bufs=4, space="PSUM") as ps_ctx:

            ident = consts.tile([P, P], F32)
            make_identity(nc, ident)
            ones1 = consts.tile([1, 64], F32)
            nc.vector.memset(ones1, 1.0)
            # cos/ssign: [S, 64] -> [128, 16*64] (s = st*128 + p)
            cos_sb = consts.tile([P, ST * DK], F32)
            nc.sync.dma_start(out=cos_sb,
                              in_=cos_d.rearrange("(t p) d -> p t d", p=P))
            ssg_sb = consts.tile([P, ST * DK], F32)
            nc.sync.dma_start(out=ssg_sb,
                              in_=ssg_d.rearrange("(t p) d -> p t d", p=P))
            if use_bias:
                bias_sb = consts.tile([4, DG], F32)
                nc.sync.dma_start(out=bias_sb, in_=bias_d[:, :])
                ones_sb = consts.tile([1, P], F32)
                nc.sync.dma_start(out=ones_sb, in_=ones_d[:, :])

            # persistent activations
            qT = [qk_pool.tile([P, S], MDT, tag="qk", name=f"qT{i}") for i in range(HPAIRS)]
            kT = [qk_pool.tile([P, S], MDT, tag="qk", name=f"kT{i}") for i in range(HPAIRS)]
            v_all = v_pool.tile([P, HG * ST * VSTRIDE], MDT)
            # ones columns of V' (single strided broadcast copy)
            ones_col = consts.tile([P, 1], F32)
            nc.vector.memset(ones_col, 1.0)
            ones_bc = bass.AP(tensor=ones_col.tensor, offset=ones_col.offset,
                              ap=[ones_col.ap[0], [0, HG], [0, ST], [0, 1]])
            nc.vector.tensor_copy(
                v_all.rearrange("p (h t c) -> p h t c", h=HG, t=ST)[:, :, :, DK:DK + 1],
                ones_bc)
            ctxT = [ctx_pool.tile([P, S], MDT, tag="ctx", name=f"ctxT{i}") for i in range(HPAIRS)]

            # ---------------- projections + RoPE + transposes --------------
            def cos_bc(st, half):
                # cos/ssign slice [128, 32] broadcast over 8 heads
                src = cos_sb if half is None else ssg_sb
                width = DK if half is None else 32
                off = st * DK + (0 if half in (None, 0) else 32)
                sl = src[:, off:off + width]
                return bass.AP(tensor=sl.tensor, offset=sl.offset,
                               ap=[sl.ap[0], [0, HG], [1, width]])

            for t_i, (x_t, w_t) in enumerate(((xqT, wqT), (xkT, wkT), (xvT, wvT))):
                for sg in range(QR):           # groups of 4 s-tiles
                    xg = [xT_pool.tile([P, 512], MDT, tag="xT", name=f"xg{i}") for i in range(CC)]
                    for cc in range(CC):
                        nc.sync.dma_start(
                            out=xg[cc],
                            in_=dcast(x_t[cc * P:(cc + 1) * P,
                                          sg * 512:(sg + 1) * 512]))
                    if sg == 0:
                        wg = [w_pool.tile([P, DG], MDT, tag="w", name=f"wg{i}") for i in range(CC)]
                        for cc in range(CC):
                            nc.sync.dma_start(
                                out=wg[cc],
                                in_=dcast(w_t[cc * P:(cc + 1) * P, :]))
                    for sti in range(4):
                        st = sg * 4 + sti
                        psum = ps_main.tile([P, DG], F32, tag="ps")
                        if use_bias:
                            nc.tensor.matmul(psum, ones_sb,
                                             bias_sb[t_i:t_i + 1, :],
                                             start=True, stop=False)
                        for cc in range(CC):
                            nc.tensor.matmul(
                                psum, mmcast(xg[cc][:, sti * P:(sti + 1) * P]),
                                mmcast(wg[cc]),
                                start=(cc == 0 and not use_bias),
                                stop=(cc == CC - 1))
                        if t_i < 2:
                            # RoPE: nat = psum*cos ; nat += shift(psum)*ssign
                            nat = nat_pool.tile([P, DG], F32, tag="nat")
                            tmp = work_pool.tile([P, DG], F32, tag="w512")
                            nat4 = nat.rearrange("p (h t d) -> p h t d", h=HG, t=2)
                            tmp4 = tmp.rearrange("p (h t d) -> p h t d", h=HG, t=2)
                            ps4 = psum.rearrange("p (h t d) -> p h t d", h=HG, t=2)
                            nc.vector.tensor_mul(
                                nat.rearrange("p (h d) -> p h d", h=HG),
                                psum.rearrange("p (h d) -> p h d", h=HG),
                                cos_bc(st, None))
                            nc.vector.tensor_mul(tmp4[:, :, 0, :], ps4[:, :, 1, :],
                                                 cos_bc(st, 0))
                            nc.vector.tensor_mul(tmp4[:, :, 1, :], ps4[:, :, 0, :],
                                                 cos_bc(st, 1))
                            nc.vector.tensor_add(nat, nat, tmp)
                            dest = qT if t_i == 0 else kT
                            for hp in range(HPAIRS):
                                pt = ps_ctx.tile([P, P], F32, tag="pc")
                                nc.tensor.transpose(
                                    pt, nat[:, hp * P:(hp + 1) * P], ident)
                                nc.vector.tensor_copy(
                                    dest[hp][:, st * P:(st + 1) * P], pt)
                        else:
                            v4 = v_all.rearrange("p (h t c) -> p h t c",
                                                 h=HG, t=ST)
                            for h in range(HG):
                                nc.vector.tensor_copy(
                                    v4[:, h, st, 0:DK],
                                    psum[:, h * DK:(h + 1) * DK])

            if phases < 2:
                for i in range(4):
                    ot = work_pool.tile([P, 512], F32, tag="w512",
                                        name=f"dump{i}")
                    nc.vector.tensor_copy(ot, qT[i][:, 0:512].bitcast(F32))
                    nc.sync.dma_start(out=out_d[i * P:(i + 1) * P, 0:512], in_=ot)
                return nc
            # ------------- attention + inlined output projection ------------
            # qr-outer so each q-range's output projection follows right
            # after its attention, giving PE dense filler work while ACT
            # works through the exps (keeps HAM warm).
            is_ge = mybir.AluOpType.is_ge
            Exp = mybir.ActivationFunctionType.Exp
            wo = {}
            if phases >= 3:
                for nr in range(2):
                    for dc in range(4):
                        wt = w_pool.tile([P, 512], MDT, tag="w",
                                         name=f"wo{nr}_{dc}")
                        nc.sync.dma_start(
                            out=wt,
                            in_=dcast(woT[dc * P:(dc + 1) * P,
                                          nr * 512:(nr + 1) * 512]))
                        wo[(nr, dc)] = wt
            for qr in range(QR):
                for hp in range(HPAIRS):
                    hA, hB = 2 * hp, 2 * hp + 1
                    pcA = ps_ctx.tile([P, 512], F32, tag="pc")
                    pcB = ps_ctx.tile([P, 512], F32, tag="pc")
                    n_kc = 4 * (qr + 1)
                    for kc in range(n_kc):
                        psA = ps_main.tile([P, 512], F32, tag="ps")
                        psB = ps_main.tile([P, 512], F32, tag="ps")
                        qsl = slice(qr * 512, (qr + 1) * 512)
                        ksl = slice(kc * P, (kc + 1) * P)
                        nc.tensor.matmul(psA, mmcast(kT[hp][0:64, ksl]),
                                         mmcast(qT[hp][0:64, qsl]),
                                         start=True, stop=True, tile_position=(0, 0))
                        nc.tensor.matmul(psB, mmcast(kT[hp][64:128, ksl]),
                                         mmcast(qT[hp][64:128, qsl]),
                                         start=True, stop=True, tile_position=(64, 0))
                        eA = work_pool.tile([P, 512], MDT, tag="w512")
                        eB = work_pool.tile([P, 512], MDT, tag="w512")
                        nc.scalar.activation(out=eA, in_=psA, func=Exp, scale=0.125)
                        nc.scalar.activation(out=eB, in_=psB, func=Exp, scale=0.125)
                        j = kc - 4 * qr
                        if j >= 0:  # diagonal block: keep qq - kk - 128*j >= 0
                            for e in (eA, eB):
                                nc.gpsimd.affine_select(
                                    out=e, in_=e, compare_op=is_ge, fill=0.0,
                                    base=-128 * j, channel_multiplier=-1,
                                    pattern=[[1, 512]])
                        v4 = v_all.rearrange("p (h t c) -> p h t c", h=HG, t=ST)
                        nc.tensor.matmul(pcA, mmcast(v4[:, hA, kc, :]), mmcast(eA),
                                         start=(kc == 0), stop=(kc == n_kc - 1))
                        nc.tensor.matmul(pcB, mmcast(v4[:, hB, kc, :]), mmcast(eB),
                                         start=(kc == 0), stop=(kc == n_kc - 1))
                    qsl = slice(qr * 512, (qr + 1) * 512)
                    denA = den_pool.tile([1, 512], F32, tag="rec", bufs=4,
                                         name="denA")
                    denB = den_pool.tile([1, 512], F32, tag="rec", bufs=4,
                                         name="denB")
                    nc.vector.tensor_copy(denA, pcA[64:65, :])
                    nc.vector.tensor_copy(denB, pcB[64:65, :])
                    pbc = ps_main.tile([P, 512], F32, tag="ps")
                    nc.tensor.matmul(pbc[0:64, :], ones1, denA,
                                     start=True, stop=True, tile_position=(0, 0),
                                     skip_group_check=True)
                    nc.tensor.matmul(pbc[64:128, :], ones1, denB,
                                     start=True, stop=True, tile_position=(0, 64),
                                     skip_group_check=True)
                    rbc = work_pool.tile([P, 512], F32, tag="w512")
                    nc.vector.reciprocal(out=rbc, in_=pbc)
                    nc.vector.tensor_mul(ctxT[hp][0:64, qsl], pcA[0:64, :],
                                         rbc[0:64, :])
                    nc.vector.tensor_mul(ctxT[hp][64:128, qsl], pcB[0:64, :],
                                         rbc[64:128, :])

                if phases >= 3:
                    for sti in range(4):
                        st = qr * 4 + sti
                        for nr in range(2):
                            po = ps_main.tile([P, 512], F32, tag="ps")
                            for dc in range(4):
                                nc.tensor.matmul(
                                    po, mmcast(ctxT[dc][:, st * P:(st + 1) * P]),
                                    mmcast(wo[(nr, dc)]),
                                    start=(dc == 0), stop=(dc == 3))
                            ot = work_pool.tile([P, 512], F32, tag="w512")
                            nc.vector.tensor_copy(ot, po)
                            nc.sync.dma_start(
                                out=out_d[st * P:(st + 1) * P,
                                          nr * 512:(nr + 1) * 512],
                                in_=ot)

            if phases < 3:
                for i in range(4):
                    ot = work_pool.tile([P, 512], F32, tag="w512",
                                        name=f"dump{i}")
                    nc.vector.tensor_copy(ot, ctxT[i][:, 0:512].bitcast(F32))
                    nc.sync.dma_start(out=out_d[i * P:(i + 1) * P, 0:512], in_=ot)
    return nc


def _build_program_v3(use_bias: bool, mm_dt: str = "fp32r"):
    """Interleaved emission: projection and output-projection PE work is
    round-robined into the attention instruction stream so the in-order
    PE has filler work while ACT computes exps (keeps HAM warm)."""
    from collections import deque

    import concourse.bass as bass
    import concourse.mybir as mybir
    import concourse.tile as tile
    from concourse.masks import make_identity

    F32 = mybir.dt.float32
    MDT = mybir.dt.float32r if mm_dt == "fp32r" else F32

    def dcast(ap):
        return ap.bitcast(MDT) if MDT is not F32 else ap

    nc = bass.Bass()
    xs = {t: nc.dram_tensor(f"x{t}T", [D, S], F32, kind="ExternalInput")
          for t in "qkv"}
    ws = {t: nc.dram_tensor(f"w{t}T", [D, DG], F32, kind="ExternalInput")
          for t in "qkv"}
    woT = nc.dram_tensor("woT", [DG, D], F32, kind="ExternalInput")
    cos_d = nc.dram_tensor("cos_d", [S, DK], F32, kind="ExternalInput")
    ssg_d = nc.dram_tensor("ssg_d", [S, DK], F32, kind="ExternalInput")
    if use_bias:
        bias_d = nc.dram_tensor("bias_d", [4, DG], F32, kind="ExternalInput")
        ones_d = nc.dram_tensor("ones_d", [1, P], F32, kind="ExternalInput")
    out_d = nc.dram_tensor("out", [S, D], F32, kind="ExternalOutput")

    with tile.TileContext(nc) as tc:
        with tc.tile_pool(name="consts", bufs=1) as consts, \
             tc.tile_pool(name="xT", bufs=6) as xT_pool, \
             tc.tile_pool(name="w", bufs=32) as w_pool, \
             tc.tile_pool(name="nat", bufs=2) as nat_pool, \
             tc.tile_pool(name="kt", bufs=4) as kt_pool, \
             tc.tile_pool(name="qt", bufs=8) as qt_pool, \
             tc.tile_pool(name="vp", bufs=1) as v_pool, \
             tc.tile_pool(name="ctx", bufs=8) as ctx_pool, \
             tc.tile_pool(name="den", bufs=1) as den_pool, \
             tc.tile_pool(name="w512", bufs=4) as work_pool, \
             tc.tile_pool(name="psm", bufs=4, space="PSUM") as ps_main, \
             tc.tile_pool(name="psb", bufs=1, space="PSUM") as ps_bc, \
             tc.tile_pool(name="psc", bufs=3, space="PSUM") as ps_ctx:

            ident = consts.tile([P, P], F32)
            make_identity(nc, ident)
            ones1 = consts.tile([1, 64], F32)
            nc.vector.memset(ones1, 1.0)
            cos_sb = consts.tile([P, ST * DK], F32)
            nc.sync.dma_start(out=cos_sb,
                              in_=cos_d.rearrange("(t p) d -> p t d", p=P))
            ssg_sb = consts.tile([P, ST * DK], F32)
            nc.sync.dma_start(out=ssg_sb,
                              in_=ssg_d.rearrange("(t p) d -> p t d", p=P))
            if use_bias:
                bias_sb = consts.tile([4, DG], F32)
                nc.sync.dma_start(out=bias_sb, in_=bias_d[:, :])
                ones_sb = consts.tile([1, P], F32)
                nc.sync.dma_start(out=ones_sb, in_=ones_d[:, :])

            kT = [kt_pool.tile([P, S], MDT, tag="kt", name=f"kT{i}")
                  for i in range(HPAIRS)]
            v_all = v_pool.tile([P, HG * ST * VSTRIDE], MDT)
            ones_col = consts.tile([P, 1], F32)
            nc.vector.memset(ones_col, 1.0)
            ones_bc = bass.AP(tensor=ones_col.tensor, offset=ones_col.offset,
                              ap=[ones_col.ap[0], [0, HG], [0, ST], [0, 1]])
            v4 = v_all.rearrange("p (h t c) -> p h t c", h=HG, t=ST)
            nc.vector.tensor_copy(v4[:, :, :, DK:DK + 1], ones_bc)

            # all weights resident
            wg = {}
            for ti, t in enumerate("qkv"):
                for cc in range(CC):
                    wt = w_pool.tile([P, DG], MDT, tag="w", name=f"w{t}{cc}")
                    nc.sync.dma_start(out=wt,
                                      in_=dcast(ws[t][cc * P:(cc + 1) * P, :]))
                    wg[(t, cc)] = wt
            wo = {}
            for nr in range(2):
                for dc in range(4):
                    wt = w_pool.tile([P, 512], MDT, tag="w",
                                     name=f"wo{nr}_{dc}")
                    nc.sync.dma_start(
                        out=wt, in_=dcast(woT[dc * P:(dc + 1) * P,
                                               nr * 512:(nr + 1) * 512]))
                    wo[(nr, dc)] = wt

            qts = {}   # (sg, hp) -> [128, 512] MDT
            ctxs = {}  # (qr, hp) -> [128, 512] MDT
            xgs = {}   # (t, sg) -> chunk list
            pending_nat = []

            def flush_transposes():
                while pending_nat:
                    ti, sg, sti, st, nat = pending_nat.pop(0)
                    for hp in range(HPAIRS):
                        pt = ps_main.tile([P, P], F32, tag="ps", name="pt")
                        nc.tensor.transpose(pt, nat[:, hp * P:(hp + 1) * P],
                                            ident)
                        if ti == 0:
                            nc.vector.tensor_copy(
                                qts[(sg, hp)][:, sti * P:(sti + 1) * P], pt)
                        else:
                            nc.vector.tensor_copy(
                                kT[hp][:, st * P:(st + 1) * P], pt)

            def cos_bc(st, half):
                src = cos_sb if half is None else ssg_sb
                width = DK if half is None else 32
                off = st * DK + (0 if half in (None, 0) else 32)
                sl = src[:, off:off + width]
                return bass.AP(tensor=sl.tensor, offset=sl.offset,
                               ap=[sl.ap[0], [0, HG], [1, width]])

            def emit_proj_dma(t, sg):
                xg = [xT_pool.tile([P, 512], MDT, tag="xT",
                                   name=f"x{t}{sg}_{i}") for i in range(CC)]
                for cc in range(CC):
                    nc.sync.dma_start(
                        out=xg[cc],
                        in_=dcast(xs[t][cc * P:(cc + 1) * P,
                                        sg * 512:(sg + 1) * 512]))
                xgs[(t, sg)] = xg

            def emit_proj_unit(ti, t, sg, sti):
                st = sg * 4 + sti
                if sti == 0 and ti == 0:
                    for hp in range(HPAIRS):
                        qts[(sg, hp)] = qt_pool.tile(
                            [P, 512], MDT, tag="qt", name=f"qt{sg}_{hp}")
                xg = xgs[(t, sg)]
                psum = ps_main.tile([P, DG], F32, tag="ps")
                if use_bias:
                    nc.tensor.matmul(psum, ones_sb, bias_sb[ti:ti + 1, :],
                                     start=True, stop=False)
                for cc in range(CC):
                    nc.tensor.matmul(psum, xg[cc][:, sti * P:(sti + 1) * P],
                                     wg[(t, cc)],
                                     start=(cc == 0 and not use_bias),
                                     stop=(cc == CC - 1))
                if ti < 2:
                    flush_transposes()
                    nat = nat_pool.tile([P, DG], F32, tag="nat")
                    tmp = work_pool.tile([P, DG], F32, tag="w512")
                    tmp4 = tmp.rearrange("p (h t d) -> p h t d", h=HG, t=2)
                    ps4 = psum.rearrange("p (h t d) -> p h t d", h=HG, t=2)
                    nc.vector.tensor_mul(
                        nat.rearrange("p (h d) -> p h d", h=HG),
                        psum.rearrange("p (h d) -> p h d", h=HG),
                        cos_bc(st, None))
                    nc.vector.tensor_mul(tmp4[:, :, 0, :], ps4[:, :, 1, :],
                                         cos_bc(st, 0))
                    nc.vector.tensor_mul(tmp4[:, :, 1, :], ps4[:, :, 0, :],
                                         cos_bc(st, 1))
                    nc.vector.tensor_add(nat, nat, tmp)
                    # transposes run one unit later (PE meets them after the
                    # in-order DVE has drained this unit's RoPE chain)
                    pending_nat.append((ti, sg, sti, st, nat))
                else:
                    for h in range(HG):
                        nc.vector.tensor_copy(v4[:, h, st, 0:DK],
                                              psum[:, h * DK:(h + 1) * DK])

            def emit_outproj_unit(qr, sti, nr):
                st = qr * 4 + sti
                po = ps_main.tile([P, 512], F32, tag="ps")
                for dc in range(4):
                    nc.tensor.matmul(po, ctxs[(qr, dc)][:, sti * P:(sti + 1) * P],
                                     wo[(nr, dc)], start=(dc == 0),
                                     stop=(dc == 3))
                ot = work_pool.tile([P, 512], F32, tag="w512")
                nc.vector.tensor_copy(ot, po)
                nc.sync.dma_start(
                    out=out_d[st * P:(st + 1) * P, nr * 512:(nr + 1) * 512],
                    in_=ot)

            is_ge = mybir.AluOpType.is_ge
            Exp = mybir.ActivationFunctionType.Exp

            # prologue: projections for s-group 0, prefetch s-group 1
            for ti, t in enumerate("qkv"):
                emit_proj_dma(t, 0)
                for sti in range(4):
                    emit_proj_unit(ti, t, 0, sti)
            for t in "qkv":
                emit_proj_dma(t, 1)

            from functools import partial
            for qr in range(QR):
                flush_transposes()
                fillers = deque()
                if qr + 1 < QR:
                    for ti, t in enumerate("qkv"):
                        for sti in range(4):
                            fillers.append(
                                partial(emit_proj_unit, ti, t, qr + 1, sti))
                if qr + 2 < QR:
                    # prefetch next-next s-group's x chunks well before use
                    for t in "qkv":
                        fillers.append(partial(emit_proj_dma, t, qr + 2))
                if qr >= 1:
                    for sti in range(4):
                        for nr in range(2):
                            fillers.append(
                                partial(emit_outproj_unit, qr - 1, sti, nr))
                n_slots = 16 * (qr + 1)
                # +HPAIRS: the deferred per-head-pair normalizes are appended
                # while the loop runs; reserve pace slots so they drain
                # interleaved instead of bursting at the qr boundary
                pace = max(1, n_slots // (len(fillers) + HPAIRS))
                cnt = 0
                for hp in range(HPAIRS):
                    hA, hB = 2 * hp, 2 * hp + 1
                    pcA = ps_ctx.tile([P, 512], F32, tag="pc")
                    pcB = ps_ctx.tile([P, 512], F32, tag="pc")
                    n_kc = 4 * (qr + 1)
                    for kc in range(n_kc):
                        psA = ps_main.tile([P, 512], F32, tag="ps")
                        psB = ps_main.tile([P, 512], F32, tag="ps")
                        ksl = slice(kc * P, (kc + 1) * P)
                        nc.tensor.matmul(psA, kT[hp][0:64, ksl],
                                         qts[(qr, hp)][0:64, :],
                                         start=True, stop=True,
                                         tile_position=(0, 0))
                        nc.tensor.matmul(psB, kT[hp][64:128, ksl],
                                         qts[(qr, hp)][64:128, :],
                                         start=True, stop=True,
                                         tile_position=(64, 0))
                        eA = work_pool.tile([P, 512], MDT, tag="w512")
                        eB = work_pool.tile([P, 512], MDT, tag="w512")
                        nc.scalar.activation(out=eA, in_=psA, func=Exp,
                                             scale=0.125)
                        nc.scalar.activation(out=eB, in_=psB, func=Exp,
                                             scale=0.125)
                        j = kc - 4 * qr
                        if j >= 0:
                            for e in (eA, eB):
                                nc.gpsimd.affine_select(
                                    out=e, in_=e, compare_op=is_ge, fill=0.0,
                                    base=-128 * j, channel_multiplier=-1,
                                    pattern=[[1, 512]])
                        nc.tensor.matmul(pcA, v4[:, hA, kc, :], eA,
                                         start=(kc == 0), stop=(kc == n_kc - 1))
                        nc.tensor.matmul(pcB, v4[:, hB, kc, :], eB,
                                         start=(kc == 0), stop=(kc == n_kc - 1))
                        cnt += 1
                        if cnt % pace == 0 and fillers:
                            fillers.popleft()()
                    for hp2 in (hA, hB):
                        pass
                    # Quick-release the ctx PSUM banks: copy out rows
                    # unnormalized, then normalize in SBUF off the PE
                    # critical path (the 3.4us DVE reciprocal otherwise
                    # stalls the next head-pair long enough to re-throttle
                    # the PE clock).
                    denA = den_pool.tile([1, 512], F32, tag="rec", bufs=3,
                                         name="denA")
                    denB = den_pool.tile([1, 512], F32, tag="rec", bufs=3,
                                         name="denB")
                    nc.scalar.copy(denA, pcA[64:65, :])
                    nc.scalar.copy(denB, pcB[64:65, :])
                    ctx = ctx_pool.tile([P, 512], MDT, tag="ctx",
                                        name=f"ctx{qr}_{hp}")
                    ctxs[(qr, hp)] = ctx
                    nc.scalar.copy(ctx[0:64, :], pcA[0:64, :])
                    nc.scalar.copy(ctx[64:128, :], pcB[0:64, :])

                    def emit_normalize(ctx=ctx, denA=denA, denB=denB):
                        # deferred: emitted a few attention slots later so
                        # the PE/DVE never stall at the head-pair boundary
                        pbc = ps_bc.tile([P, 512], F32, tag="pbc",
                                         name="pbc")
                        nc.tensor.matmul(pbc[0:64, :], ones1, denA,
                                         start=True, stop=True,
                                         tile_position=(0, 0),
                                         skip_group_check=True)
                        nc.tensor.matmul(pbc[64:128, :], ones1, denB,
                                         start=True, stop=True,
                                         tile_position=(0, 64),
                                         skip_group_check=True)
                        rbc = work_pool.tile([P, 512], F32, tag="rbc",
                                             bufs=2, name="rbc")
                        nc.vector.reciprocal(out=rbc, in_=pbc)
                        nc.gpsimd.tensor_mul(ctx[0:64, :], ctx[0:64, :],
                                             rbc[0:64, :])
                        nc.gpsimd.tensor_mul(ctx[64:128, :], ctx[64:128, :],
                                             rbc[64:128, :])

                    fillers.append(emit_normalize)
                while fillers:
                    fillers.popleft()()
            for sti in range(4):
                for nr in range(2):
                    emit_outproj_unit(QR - 1, sti, nr)
    return nc


def _build_program_v4():
    """fp16 pipeline. Differences vs v3:
      - all matmul operands fp16 (host converts inputs): PE streaming and
        LDWEIGHTS are SBUF-bandwidth-bound, so 2-byte operands halve both.
      - Q/K projections computed directly transposed (lhsT = W chunk,
        rhs = xT chunk) -> no PE transposes at all.
      - RoPE in [d, s] layout: partition-rotated copy via 4 SBUF->SBUF DMAs,
        then 3 fp16 DVE ops (2x mode) fused with the qT/kT write.
      - causal diagonal blocks: QK matmul + exp trimmed to the unmasked
        column range; masked eAB columns zeroed by DVE memset; triangle
        masked by DVE multiply with a constant [128,128] fp16 tile
        (gpsimd affine_select dropped).
      - exp bias -2 so e^s fits fp16 (denominator carries the same factor,
        cancels in normalization).
      - V' ones column = 1/256 so rbc=256/den stays in fp16 normal range;
        the 256 is divided out of W_o on the host.
      - ctx quick-released from PSUM as fp32 (pre-normalization values can
        exceed fp16 max), normalized into fp16 off the critical path.
    """
    from collections import deque
    from functools import partial

    import concourse.bass as bass
    import concourse.mybir as mybir
    import concourse.tile as tile

    F32 = mybir.dt.float32
    F16 = mybir.dt.float16

    nc = bass.Bass()
    xs = {t: nc.dram_tensor(f"x{t}T", [D, S], F16, kind="ExternalInput")
          for t in "qkv"}
    ws = {t: nc.dram_tensor(f"w{t}T", [D, DG], F16, kind="ExternalInput")
          for t in "qkv"}
    woT = nc.dram_tensor("woT", [DG, D], F16, kind="ExternalInput")
    cosT_d = nc.dram_tensor("cosT", [P, S], F16, kind="ExternalInput")
    ssgT_d = nc.dram_tensor("ssgT", [P, S], F16, kind="ExternalInput")
    out_d = nc.dram_tensor("out", [S, D], F16, kind="ExternalOutput")

    is_ge = mybir.AluOpType.is_ge
    Exp = mybir.ActivationFunctionType.Exp
    Ln = mybir.ActivationFunctionType.Ln

    with tile.TileContext(nc) as tc:
        with tc.tile_pool(name="consts", bufs=1) as consts, \
             tc.tile_pool(name="xT", bufs=6) as xT_pool, \
             tc.tile_pool(name="w", bufs=4) as w_pool, \
             tc.tile_pool(name="nat", bufs=4) as nat_pool, \
             tc.tile_pool(name="qk", bufs=2) as qk_pool, \
             tc.tile_pool(name="vp", bufs=1) as v_pool, \
             tc.tile_pool(name="ctx", bufs=8) as ctx_pool, \
             tc.tile_pool(name="cxr", bufs=4) as cxr_pool, \
             tc.tile_pool(name="den", bufs=1) as den_pool, \
             tc.tile_pool(name="w512", bufs=5) as work_pool, \
             tc.tile_pool(name="psm", bufs=5, space="PSUM") as ps_main, \
             tc.tile_pool(name="psc", bufs=3, space="PSUM") as ps_ctx:

            xgs = {}    # (t, sg) -> chunk list
            ctxs = {}   # (qr, hp) -> [128, 512] F16 (final, normalized)
            pending_rope = []
            wg = {}
            wo = {}

            def emit_x_dma(t, sg, split=1):
                xg = xT_pool.tile([P, CC * 512], F16, tag="xT",
                                  name=f"x{t}{sg}")
                xr = xs[t].rearrange("(c p) s -> p c s",
                                     p=P)[:, :, sg * 512:(sg + 1) * 512]
                step = CC // split
                for i in range(split):
                    nc.sync.dma_start(
                        out=xg[:, i * step * 512:(i + 1) * step * 512],
                        in_=xr[:, i * step:(i + 1) * step, :])
                xgs[(t, sg)] = xg

            def emit_w_dma(t, split=1):
                wt = w_pool.tile([P, CC * DG], F16, tag="w", name=f"w{t}")
                wr = ws[t].rearrange("(c p) d -> p c d", p=P)
                step = CC // split
                for i in range(split):
                    nc.sync.dma_start(
                        out=wt[:, i * step * DG:(i + 1) * step * DG],
                        in_=wr[:, i * step:(i + 1) * step, :])
                wg[t] = wt

            # startup ordering: the first projection unit only needs xq + wq,
            # so those DMAs go first; everything else queues behind them.
            emit_x_dma("q", 0, split=4)
            emit_w_dma("q", split=4)
            emit_x_dma("k", 0)
            emit_w_dma("k")
            emit_x_dma("v", 0)
            emit_w_dma("v")

            bias_m2 = consts.tile([P, 1], F32)
            nc.vector.memset(bias_m2, -2.0)
            # lower-triangle-inclusive mask (keep q >= k), fp16
            tri32 = consts.tile([P, P], F32)
            nc.vector.memset(tri32, 1.0)
            nc.gpsimd.affine_select(out=tri32, in_=tri32, compare_op=is_ge,
                                    fill=0.0, base=0, channel_multiplier=-1,
                                    pattern=[[1, P]])
            tri = consts.tile([P, P], F16)
            nc.vector.tensor_copy(tri, tri32)
            tri_bc = bass.AP(tensor=tri.tensor, offset=tri.offset,
                             ap=[tri.ap[0], [0, 2], [1, P]])
            cosT_sb = consts.tile([P, S], F16)
            nc.sync.dma_start(out=cosT_sb, in_=cosT_d[:, :])
            ssgT_sb = consts.tile([P, S], F16)
            nc.sync.dma_start(out=ssgT_sb, in_=ssgT_d[:, :])

            # persistent activations
            qT_all = qk_pool.tile([P, HPAIRS * S], F16, tag="qk", name="qT")
            kT_all = qk_pool.tile([P, HPAIRS * S], F16, tag="qk", name="kT")
            v_all = v_pool.tile([P, HG * ST * VPACK], F16)
            v4 = v_all.rearrange("p (h t c) -> p h t c", h=HG, t=ST)
            ones_col = consts.tile([P, 1], F16)
            nc.vector.memset(ones_col, 1.0 / 256.0)
            ones_bc = bass.AP(tensor=ones_col.tensor, offset=ones_col.offset,
                              ap=[ones_col.ap[0], [0, HG], [0, ST], [0, DK]])
            nc.vector.tensor_copy(v4[:, :, :, DK:2 * DK], ones_bc)

            wo_all = w_pool.tile([P, 4 * D], F16, tag="w", name="wo")
            nc.sync.dma_start(out=wo_all,
                              in_=woT.rearrange("(dc p) o -> p dc o", p=P))
            for nr in range(2):
                for dc in range(4):
                    wo[(nr, dc)] = wo_all[:, dc * D + nr * 512:
                                          dc * D + (nr + 1) * 512]

            def flush_rope(keep=0):
                while len(pending_rope) > keep:
                    t, hp, sg, nat0, nat0r = pending_rope.pop(0)
                    dest = qT_all if t == "q" else kT_all
                    ssl = slice(sg * 512, (sg + 1) * 512)
                    tmp1 = work_pool.tile([P, 512], F16, tag="rp", bufs=4,
                                          name="rp1")
                    tmp2 = work_pool.tile([P, 512], F16, tag="rp", bufs=4,
                                          name="rp2")
                    nc.vector.tensor_mul(tmp1, nat0, cosT_sb[:, ssl])
                    nc.vector.tensor_mul(tmp2, nat0r, ssgT_sb[:, ssl])
                    nc.vector.tensor_add(
                        dest[:, hp * S + sg * 512:hp * S + (sg + 1) * 512],
                        tmp1, tmp2)

            def emit_qk_unit(t, hp, sg):
                flush_rope(keep=1)
                xg = xgs[(t, sg)]
                psum = ps_main.tile([P, 512], F32, tag="ps")
                for cc in range(CC):
                    nc.tensor.matmul(
                        psum,
                        wg[t][:, cc * DG + hp * P:cc * DG + (hp + 1) * P],
                        xg[:, cc * 512:(cc + 1) * 512],
                        start=(cc == 0), stop=(cc == CC - 1))
                nat0 = nat_pool.tile([P, 512], F16, tag="nat", name="nat0")
                nc.vector.tensor_copy(nat0, psum)
                nat0r = nat_pool.tile([P, 512], F16, tag="nat", name="nat0r")
                for (dst, src) in ((0, 32), (32, 0), (64, 96), (96, 64)):
                    nc.sync.dma_start(out=nat0r[dst:dst + 32, :],
                                      in_=nat0[src:src + 32, :])
                pending_rope.append((t, hp, sg, nat0, nat0r))

            def emit_v_unit(sg, sti):
                flush_rope(keep=1)
                st = sg * 4 + sti
                xg = xgs[("v", sg)]
                psum = ps_main.tile([P, DG], F32, tag="ps")
                for cc in range(CC):
                    nc.tensor.matmul(
                        psum,
                        xg[:, cc * 512 + sti * P:cc * 512 + (sti + 1) * P],
                        wg["v"][:, cc * DG:(cc + 1) * DG],
                        start=(cc == 0), stop=(cc == CC - 1))
                ps3 = psum.rearrange("p (h c) -> p h c", h=HG)
                nc.vector.tensor_copy(v4[:, :, st, 0:DK], ps3)

            def emit_outproj_unit(qr, sti, nr):
                flush_rope(keep=1)
                st = qr * 4 + sti
                po = ps_main.tile([P, 512], F32, tag="ps")
                for dc in range(4):
                    nc.tensor.matmul(po,
                                     ctxs[(qr, dc)][:, sti * P:(sti + 1) * P],
                                     wo[(nr, dc)], start=(dc == 0),
                                     stop=(dc == 3))
                ot = work_pool.tile([P, 512], F16, tag="ot", bufs=4,
                                    name="ot")
                nc.vector.tensor_copy(ot, po)
                nc.sync.dma_start(
                    out=out_d[st * P:(st + 1) * P, nr * 512:(nr + 1) * 512],
                    in_=ot)

            # prologue: projections for s-group 0, prefetch s-group 1
            for hp in range(HPAIRS):
                emit_qk_unit("q", hp, 0)
            for hp in range(HPAIRS):
                emit_qk_unit("k", hp, 0)
            for sti in range(4):
                emit_v_unit(0, sti)
            for t in "qkv":
                emit_x_dma(t, 1)

            def emit_qk_pair(qr_, hp, kc):
                j = kc - 4 * qr_
                lo = 128 * j if j > 0 else 0
                koff = hp * S
                qoff = hp * S + qr_ * 512
                psA = ps_main.tile([P, 512], F32, tag="ps")
                psB = ps_main.tile([P, 512], F32, tag="ps")
                ksl = slice(koff + kc * P, koff + (kc + 1) * P)
                nc.tensor.matmul(psA[:, lo:512],
                                 kT_all[0:64, ksl],
                                 qT_all[0:64, qoff + lo:qoff + 512],
                                 start=True, stop=True,
                                 tile_position=(0, 0))
                nc.tensor.matmul(psB[:, lo:512],
                                 kT_all[64:128, ksl],
                                 qT_all[64:128, qoff + lo:qoff + 512],
                                 start=True, stop=True,
                                 tile_position=(64, 0))
                return psA, psB

            pend = None
            for qr in range(QR):
                flush_rope(keep=0)
                fillers = deque()
                if qr + 1 < QR:
                    for hp in range(HPAIRS):
                        fillers.append(partial(emit_qk_unit, "q", hp, qr + 1))
                    for hp in range(HPAIRS):
                        fillers.append(partial(emit_qk_unit, "k", hp, qr + 1))
                    for sti in range(4):
                        fillers.append(partial(emit_v_unit, qr + 1, sti))
                if qr + 2 < QR:
                    for t in "qkv":
                        fillers.append(partial(emit_x_dma, t, qr + 2))
                if qr >= 1:
                    for sti in range(4):
                        for nr in range(2):
                            fillers.append(
                                partial(emit_outproj_unit, qr - 1, sti, nr))
                n_slots = 16 * (qr + 1)
                pace = max(1, n_slots // (len(fillers) + HPAIRS))
                cnt = 0
                n_kc = 4 * (qr + 1)

                # software pipeline: QK(kc+1) is emitted between exp(kc) and
                # attn@V(kc) so the PE streams scores while ACT runs the exps
                if pend is None:
                    pend = emit_qk_pair(qr, 0, 0)
                for hp in range(HPAIRS):
                    hA, hB = 2 * hp, 2 * hp + 1
                    pcA = ps_ctx.tile([P, 512], F32, tag="pc")
                    pcB = ps_ctx.tile([P, 512], F32, tag="pc")
                    for kc in range(n_kc):
                        j = kc - 4 * qr
                        lo = 128 * j if j > 0 else 0
                        psA, psB = pend
                        eAB = work_pool.tile([P, 1024], F16, tag="w512")
                        e2 = eAB.rearrange("p (h c) -> p h c", h=2)
                        nc.scalar.activation(out=eAB[:, lo:512],
                                             in_=psA[:, lo:512], func=Exp,
                                             scale=0.125, bias=bias_m2)
                        nc.scalar.activation(out=eAB[:, 512 + lo:1024],
                                             in_=psB[:, lo:512], func=Exp,
                                             scale=0.125, bias=bias_m2)
                        if j >= 1:
                            nc.gpsimd.memset(e2[:, :, 0:lo], 0.0)
                        if j >= 0:
                            nc.gpsimd.tensor_mul(e2[:, :, lo:lo + P],
                                                 e2[:, :, lo:lo + P], tri_bc)
                        if kc + 1 < n_kc:
                            pend = emit_qk_pair(qr, hp, kc + 1)
                        elif hp + 1 < HPAIRS:
                            pend = emit_qk_pair(qr, hp + 1, 0)
                        elif qr + 1 < QR:
                            # cross-qr: keep the PE fed through the boundary
                            pend = emit_qk_pair(qr + 1, 0, 0)
                        nc.tensor.matmul(pcA, v4[:, hA, kc, :],
                                         eAB[:, 0:512],
                                         start=(kc == 0), stop=(kc == n_kc - 1))
                        nc.tensor.matmul(pcB, v4[:, hB, kc, :],
                                         eAB[:, 512:1024],
                                         start=(kc == 0), stop=(kc == n_kc - 1))
                        cnt += 1
                        if cnt % pace == 0 and fillers:
                            fillers.popleft()()
                    # quick-release pc: fp32 raw ctx copies (pool) + fp16
                    # reciprocal of the scaled denominator row (dve)
                    # quick-release pc, then normalize with DVE divide
                    ctx_raw = cxr_pool.tile([P, 512], F32, tag="cxr",
                                            name=f"cxr{qr}_{hp}")
                    nc.vector.tensor_copy(ctx_raw[0:64, :], pcA[0:64, :])
                    nc.vector.tensor_copy(ctx_raw[64:128, :], pcB[0:64, :])
                    lnd = den_pool.tile([P, 512], F32, tag="rec", bufs=4,
                                        name="lnd")
                    nc.scalar.activation(out=lnd[0:64, :],
                                         in_=pcA[64:128, :], func=Ln)
                    nc.scalar.activation(out=lnd[64:128, :],
                                         in_=pcB[64:128, :], func=Ln)
                    ctx = ctx_pool.tile([P, 512], F16, tag="ctx",
                                        name=f"ctx{qr}_{hp}")
                    ctxs[(qr, hp)] = ctx

                    def emit_normalize(ctx=ctx, ctx_raw=ctx_raw, lnd=lnd):
                        rbc = den_pool.tile([P, 512], F32, tag="rbc", bufs=4,
                                            name="rbc")
                        nc.scalar.activation(out=rbc[0:64, :],
                                             in_=lnd[0:64, :],
                                             func=Exp, scale=-1.0)
                        nc.scalar.activation(out=rbc[64:128, :],
                                             in_=lnd[64:128, :],
                                             func=Exp, scale=-1.0)
                        nc.vector.tensor_mul(ctx[0:64, :], ctx_raw[0:64, :],
                                             rbc[0:64, :])
                        nc.vector.tensor_mul(ctx[64:128, :],
                                             ctx_raw[64:128, :],
                                             rbc[64:128, :])

                    fillers.append(emit_normalize)
                while fillers:
                    fillers.popleft()()
            # final outproj: dc0-2 partials first (need only head-pairs
            # 0-2's ctx) so they overlap the last pair's normalize chain
            for wave in ((0, 1), (2, 3)):
                pos = []
                for sti in wave:
                    for nr in range(2):
                        po = ps_main.tile([P, 512], F32, tag="ps")
                        sl = slice(sti * P, (sti + 1) * P)
                        for dc in range(3):
                            nc.tensor.matmul(po, ctxs[(QR - 1, dc)][:, sl],
                                             wo[(nr, dc)],
                                             start=(dc == 0), stop=False)
                        pos.append((po, sti, nr))
                for po, sti, nr in pos:
                    st = (QR - 1) * 4 + sti
                    nc.tensor.matmul(po,
                                     ctxs[(QR - 1, 3)][:, sti * P:
                                                       (sti + 1) * P],
                                     wo[(nr, 3)], start=False, stop=True)
                    ot = work_pool.tile([P, 512], F16, tag="ot", bufs=4,
                                        name="ot")
                    nc.vector.tensor_copy(ot, po)
                    nc.sync.dma_start(
                        out=out_d[st * P:(st + 1) * P,
                                  nr * 512:(nr + 1) * 512],
                        in_=ot)
            flush_rope(keep=0)
    return nc


_PROG_CACHE = {}


def _get_program(use_bias: bool):
    ver = os.environ.get("KERNEL_V", "4")
    if use_bias:
        ver = "3"  # v4 has no bias path; graded problem has zero biases
    mm_dt = os.environ.get("KERNEL_MM_DT", "fp32r")
    key = (use_bias, mm_dt, ver)
    if key not in _PROG_CACHE:
        if ver == "4":
            _PROG_CACHE[key] = _build_program_v4()
        elif ver == "3":
            _PROG_CACHE[key] = _build_program_v3(use_bias, mm_dt=mm_dt)
        else:
            _PROG_CACHE[key] = _build_program(use_bias, mm_dt=mm_dt)
    return _PROG_CACHE[key]


def _rope_tables():
    inv = 1.0 / (ROPE_BASE ** (np.arange(0, DK, 2, dtype=np.float32) / DK))
    t = np.arange(S, dtype=np.float32)
    fr = t[:, None] * inv[None, :]                      # [S, 32]
    emb = np.concatenate([fr, fr], axis=-1)             # [S, 64]
    cos = np.cos(emb).astype(np.float32)
    sin = np.sin(emb).astype(np.float32)
    ssg = sin.copy()
    ssg[:, :32] = -sin[:, :32]
    return cos, ssg


def _rope_tables_T():
    """[128, S] fp16 tables in transposed head-pair layout: row p covers
    head-in-pair p//64, dim d = p%64 (both heads share the tables).
    ssgT carries the rotate-half sign (rows d<32 negative)."""
    inv = 1.0 / (ROPE_BASE ** (np.arange(0, DK, 2, dtype=np.float32) / DK))
    t = np.arange(S, dtype=np.float32)
    fr = t[:, None] * inv[None, :]                      # [S, 32]
    emb = np.concatenate([fr, fr], axis=-1)             # [S, 64]
    cosT = np.cos(emb).T                                # [64, S]
    sinT = np.sin(emb).T
    ssgT = sinT.copy()
    ssgT[:32, :] = -sinT[:32, :]
    cosT = np.concatenate([cosT, cosT], axis=0)         # [128, S]
    ssgT = np.concatenate([ssgT, ssgT], axis=0)
    return cosT.astype(np.float16), ssgT.astype(np.float16)


def kernel(query, key, value, W_q, b_q, W_k, b_k, W_v, b_v, W_o, b_o):
    _install_patches()
    from concourse.bass_utils import run_bass_kernel_spmd

    query = np.asarray(query, dtype=np.float32)
    key = np.asarray(key, dtype=np.float32)
    value = np.asarray(value, dtype=np.float32)
    W_q, W_k, W_v, W_o = (np.asarray(w, dtype=np.float32)
                          for w in (W_q, W_k, W_v, W_o))
    b_q, b_k, b_v, b_o = (np.asarray(b, dtype=np.float32)
                          for b in (b_q, b_k, b_v, b_o))

    use_bias = bool(np.any(b_q) or np.any(b_k) or np.any(b_v))
    nc = _get_program(use_bias)
    ver = "3" if use_bias else os.environ.get("KERNEL_V", "4")

    in_maps = []
    if ver == "4":
        cosT, ssgT = _rope_tables_T()
        f16 = np.float16
        for c in range(N_CORES):
            b, g = divmod(c, 2)
            gs = slice(g * DG, (g + 1) * DG)
            m = {
                "xqT": query[b].T.astype(f16),
                "xkT": key[b].T.astype(f16),
                "xvT": value[b].T.astype(f16),
                "wqT": W_q[gs, :].T.astype(f16),
                "wkT": W_k[gs, :].T.astype(f16),
                "wvT": W_v[gs, :].T.astype(f16),
                "woT": (W_o[:, gs].T / 256.0).astype(f16),
                "cosT": cosT,
                "ssgT": ssgT,
            }
            in_maps.append(m)
    else:
        cos, ssg = _rope_tables()
        for c in range(N_CORES):
            b, g = divmod(c, 2)
            gs = slice(g * DG, (g + 1) * DG)
            m = {
                "xqT": np.ascontiguousarray(query[b].T),
                "xkT": np.ascontiguousarray(key[b].T),
                "xvT": np.ascontiguousarray(value[b].T),
                "wqT": np.ascontiguousarray(W_q[gs, :].T),
                "wkT": np.ascontiguousarray(W_k[gs, :].T),
                "wvT": np.ascontiguousarray(W_v[gs, :].T),
                "woT": np.ascontiguousarray(W_o[:, gs].T),
                "cos_d": cos,
                "ssg_d": ssg,
            }
            if use_bias:
                m["bias_d"] = np.stack([b_q[gs], b_k[gs], b_v[gs],
                                        np.zeros(DG, np.float32)])
                m["ones_d"] = np.ones((1, P), np.float32)
            in_maps.append(m)

    trace = bool(int(os.environ.get("KERNEL_TRACE", "0")))
    trace_cores = None
    if trace:
        tc_env = os.environ.get("KERNEL_TRACE_CORES", "")
        trace_cores = ([int(x) for x in tc_env.split(",") if x != ""]
                       if tc_env else list(range(N_CORES)))
    try:
        res = run_bass_kernel_spmd(nc, in_maps, core_ids=list(range(N_CORES)),
                                   trace=trace, trace_cores=trace_cores)
    except Exception:
        if not trace:
            raise
        res = run_bass_kernel_spmd(nc, in_maps, core_ids=list(range(N_CORES)),
                                   trace=False)
    kernel._last_results = res

    out = np.empty((B, S, D), np.float32)
    for b in range(B):
        out[b] = (res.results[2 * b]["out"].astype(np.float32)
                  + res.results[2 * b + 1]["out"].astype(np.float32) + b_o)
    return out

